# revision 10
# baseline (speedup 1.0000x reference)
"""Event-to-image scatter kernel for Trainium2 (Bass/Tile), 8-core SPMD.

Problem: x [16, 500000, 4] f32 events (t, x, y, p) -> [16, 720, 1280, 3] f32.
Per batch: ch0 = 255 except 0 where last event at pixel has p==1; ch1 = 255
except 0 where last p==0; ch2 = ch0 + ch1. Last-write-wins on duplicate
pixels (event order).

Sharding: pure data parallel - batch dim across 8 cores (2 batches/core).

Device algorithm per batch:
  1. Decode (DVE) in 6 chunks of 651 columns over the TRANSPOSED [128, 3906]
     event layout (event n at partition n%128, column n//128, so column-major
     stream order == memory event order): pixel offset q = floor(y)*1280 +
     floor(x) (exact floor via rne-int + is_gt correction) -> offs int32;
     value v = 2*n + pol + 2 -> vals int32 (idx-encoded: max over values
     at a pixel == last event; 0 = untouched).
  2. Scatter: one 128-descriptor indirect SWDGE DMA per column (= 128
     consecutive events). Within an image, instruction order == event order
     (per-queue rings drain in order), so last-write-wins holds there;
     across the per-queue image rotation the idx-encoded values make the
     dense-pass elementwise max reproduce exact last-write-wins. Columns
     fan out over 2 queues per batch and rotate over 2 images per queue so
     Tile's WAW chains never stall the queue.
  3. Dense pass: v = max over the batch's 4 images; touched = v>0,
     pol = v&1; build the three channels, write interleaved [720, 1280, 3].
"""
import numpy as np

W, H = 1280, 720
B, N = 16, 500000
NPIX = H * W
P = 128
S = 3906            # events per partition (128*3906 = 499968)
TAIL = N - P * S    # 32
NCORES = 8
BPC = B // NCORES   # batches per core

CH = 651            # decode chunk: 6 chunks of 651 = 3906 columns
NCH = S // CH
NQPB = 2            # queues per batch
EO = 2              # image rotation depth per queue
NIMG = NQPB * EO    # images per batch

_compiled = None


def _build():
    from concourse import bacc, bass, mybir, tile

    nc = bacc.Bacc("TRN2", target_bir_lowering=False, debug=False,
                   num_swdge_queues=4)
    x_d = nc.dram_tensor("x", [BPC, N, 4], mybir.dt.float32, kind="ExternalInput")
    out_d = nc.dram_tensor("out", [BPC, H, W, 3], mybir.dt.float32,
                           kind="ExternalOutput")
    imgs = [[nc.dram_tensor(f"img{b}_{j}", [NPIX, 1], mybir.dt.int32)
             for j in range(NIMG)] for b in range(BPC)]

    f32, i32 = mybir.dt.float32, mybir.dt.int32

    with tile.TileContext(nc) as tc:
        with tc.tile_pool(name="sbuf", bufs=2) as pool, \
             tc.tile_pool(name="persist", bufs=1) as pp:
            ztile = pp.tile([P, 1200], i32)
            nc.vector.memset(ztile[:], 0)
            for b in range(BPC):
                for j in range(NIMG):
                    for k in range(6):
                        nc.sync.dma_start(
                            out=imgs[b][j].ap()[k * 153600:(k + 1) * 153600, :]
                            .rearrange("(p f) o -> p (f o)", p=P),
                            in_=ztile[:],
                        )

            # vbase[p, s] = 2*(s*128 + p) + 2  (f32-exact: max < 2^21)
            vbase = pp.tile([P, S], f32)
            nc.gpsimd.iota(vbase[:], pattern=[[2 * P, S]], base=2,
                           channel_multiplier=2,
                           allow_small_or_imprecise_dtypes=True)
            tvbase = pp.tile([TAIL, 1], f32)
            nc.gpsimd.iota(tvbase[:], pattern=[[0, 1]], base=2 * P * S + 2,
                           channel_multiplier=2,
                           allow_small_or_imprecise_dtypes=True)

            last = [[None] * NIMG for b in range(BPC)]

            for b in range(BPC):
                src = x_d.ap()[b, :P * S, :].rearrange("(s p) f -> p s f", p=P)
                for ch in range(NCH):
                    e0 = ch * CH
                    raw = pool.tile([P, CH * 4], f32, tag="raw")
                    nc.sync.dma_start(out=raw[:], in_=src[:, e0:e0 + CH, :])
                    xs, ys, ps = raw[:, 1::4], raw[:, 2::4], raw[:, 3::4]
                    fx = pool.tile([P, CH], f32, tag="fx")
                    fy = pool.tile([P, CH], f32, tag="fy")
                    ti = pool.tile([P, CH], i32, tag="ti")
                    tg = pool.tile([P, CH], f32, tag="tg")
                    offs_c = pool.tile([P, CH], i32, tag="offs")
                    vals_c = pool.tile([P, CH], i32, tag="vals")
                    # exact floor(xs)
                    nc.vector.tensor_copy(out=ti[:], in_=xs)
                    nc.vector.tensor_copy(out=fx[:], in_=ti[:])
                    nc.vector.tensor_tensor(out=tg[:], in0=fx[:], in1=xs,
                                            op=mybir.AluOpType.is_gt)
                    nc.vector.tensor_sub(out=fx[:], in0=fx[:], in1=tg[:])
                    # exact floor(ys)
                    nc.vector.tensor_copy(out=ti[:], in_=ys)
                    nc.vector.tensor_copy(out=fy[:], in_=ti[:])
                    nc.vector.tensor_tensor(out=tg[:], in0=fy[:], in1=ys,
                                            op=mybir.AluOpType.is_gt)
                    nc.vector.tensor_sub(out=fy[:], in0=fy[:], in1=tg[:])
                    # q = fy*1280 + fx -> offs int32
                    nc.vector.tensor_scalar(out=fy[:], in0=fy[:],
                                            scalar1=float(W), scalar2=None,
                                            op0=mybir.AluOpType.mult)
                    nc.vector.tensor_add(out=fy[:], in0=fy[:], in1=fx[:])
                    nc.vector.tensor_copy(out=offs_c[:], in_=fy[:])
                    # v = vbase + pol -> vals int32
                    nc.vector.tensor_add(out=tg[:], in0=vbase[:, e0:e0 + CH],
                                         in1=ps)
                    nc.vector.tensor_copy(out=vals_c[:], in_=tg[:])

                    # scatter: one 128-desc instruction per column
                    for c in range(CH):
                        gc = e0 + c
                        q = (gc & 1)
                        j = q * EO + ((gc >> 1) % EO)
                        h = nc.gpsimd.indirect_dma_start(
                            out=imgs[b][j].ap(),
                            out_offset=bass.IndirectOffsetOnAxis(
                                ap=offs_c[:, c:c + 1], axis=0),
                            in_=vals_c[:, c:c + 1],
                            in_offset=None,
                        )
                        qn = b * NQPB + q
                        if qn:
                            h.ins.queue = f"qPoolDynamic{qn}"
                        last[b][j] = h.ins

                # tail: events 499968..500000 (32) -> partitions 0..31
                toff = pp.tile([TAIL, 1], i32, name=f"toff{b}")
                tval = pp.tile([TAIL, 1], i32, name=f"tval{b}")
                traw = pool.tile([TAIL, 4], f32, tag="traw")
                nc.sync.dma_start(out=traw[:],
                                  in_=x_d.ap()[b, P * S:P * S + TAIL, :])
                txs, tys, tps = traw[:, 1:2], traw[:, 2:3], traw[:, 3:4]
                tfx = pool.tile([TAIL, 1], f32, tag="tfx")
                tfy = pool.tile([TAIL, 1], f32, tag="tfy")
                tti = pool.tile([TAIL, 1], i32, tag="tti")
                ttg = pool.tile([TAIL, 1], f32, tag="ttg")
                nc.vector.tensor_copy(out=tti[:], in_=txs)
                nc.vector.tensor_copy(out=tfx[:], in_=tti[:])
                nc.vector.tensor_tensor(out=ttg[:], in0=tfx[:], in1=txs,
                                        op=mybir.AluOpType.is_gt)
                nc.vector.tensor_sub(out=tfx[:], in0=tfx[:], in1=ttg[:])
                nc.vector.tensor_copy(out=tti[:], in_=tys)
                nc.vector.tensor_copy(out=tfy[:], in_=tti[:])
                nc.vector.tensor_tensor(out=ttg[:], in0=tfy[:], in1=tys,
                                        op=mybir.AluOpType.is_gt)
                nc.vector.tensor_sub(out=tfy[:], in0=tfy[:], in1=ttg[:])
                nc.vector.tensor_scalar(out=tfy[:], in0=tfy[:],
                                        scalar1=float(W), scalar2=None,
                                        op0=mybir.AluOpType.mult)
                nc.vector.tensor_add(out=tfy[:], in0=tfy[:], in1=tfx[:])
                nc.vector.tensor_copy(out=toff[:], in_=tfy[:])
                nc.vector.tensor_add(out=ttg[:], in0=tvbase[:], in1=tps)
                nc.vector.tensor_copy(out=tval[:], in_=ttg[:])
                h = nc.gpsimd.indirect_dma_start(
                    out=imgs[b][0].ap(),
                    out_offset=bass.IndirectOffsetOnAxis(ap=toff[:, :1], axis=0),
                    in_=tval[:, :1],
                    in_offset=None,
                )
                if b * NQPB:
                    h.ins.queue = f"qPoolDynamic{b * NQPB}"
                last[b][0] = h.ins

            # dense conversion: max over images -> touched/pol -> [H, W, 3]
            for b in range(BPC):
                for t in range(6):
                    rows = 128 if t < 5 else 80
                    r0 = t * 128
                    a_i = pool.tile([P, W], i32, tag="ai")
                    b_i = pool.tile([P, W], i32, tag="bi")
                    m1 = pool.tile([P, W], i32, tag="m1")
                    tt = pool.tile([P, W], i32, tag="tt")
                    mf = pool.tile([P, W], f32, tag="mf")
                    ot = pool.tile([P, 3 * W], f32, tag="ot")
                    for j in range(NIMG):
                        dst = a_i if j == 0 else b_i
                        ld = nc.sync.dma_start(
                            out=dst[:rows, :],
                            in_=imgs[b][j].ap()[r0 * W:(r0 + rows) * W, :]
                            .rearrange("(p f) o -> p (f o)", p=rows),
                        )
                        tile.add_dep_helper(ld.ins, last[b][j],
                                            reason="scatter before read")
                        if j:
                            nc.vector.tensor_tensor(
                                out=a_i[:rows, :], in0=a_i[:rows, :],
                                in1=b_i[:rows, :], op=mybir.AluOpType.max)
                    # t = v>0; pol = v&1; m1 = t*pol; m0 = t - m1 (in tt)
                    nc.vector.tensor_scalar(out=tt[:rows, :], in0=a_i[:rows, :],
                                            scalar1=0, scalar2=None,
                                            op0=mybir.AluOpType.is_gt)
                    nc.vector.tensor_scalar(out=m1[:rows, :], in0=a_i[:rows, :],
                                            scalar1=1, scalar2=None,
                                            op0=mybir.AluOpType.bitwise_and)
                    # ch2 = 510 - 255*t  (write before tt is clobbered)
                    nc.vector.tensor_copy(out=mf[:rows, :], in_=tt[:rows, :])
                    nc.vector.tensor_scalar(out=ot[:rows, 2::3],
                                            in0=mf[:rows, :],
                                            scalar1=-255.0, scalar2=510.0,
                                            op0=mybir.AluOpType.mult,
                                            op1=mybir.AluOpType.add)
                    nc.vector.tensor_sub(out=tt[:rows, :], in0=tt[:rows, :],
                                         in1=m1[:rows, :])
                    # ch0 = 255 - 255*m1; ch1 = 255 - 255*m0
                    nc.vector.tensor_copy(out=mf[:rows, :], in_=m1[:rows, :])
                    nc.vector.tensor_scalar(out=ot[:rows, 0::3],
                                            in0=mf[:rows, :],
                                            scalar1=-255.0, scalar2=255.0,
                                            op0=mybir.AluOpType.mult,
                                            op1=mybir.AluOpType.add)
                    nc.vector.tensor_copy(out=mf[:rows, :], in_=tt[:rows, :])
                    nc.vector.tensor_scalar(out=ot[:rows, 1::3],
                                            in0=mf[:rows, :],
                                            scalar1=-255.0, scalar2=255.0,
                                            op0=mybir.AluOpType.mult,
                                            op1=mybir.AluOpType.add)
                    nc.sync.dma_start(
                        out=out_d.ap()[b, r0:r0 + rows, :, :]
                        .rearrange("p w c -> p (w c)"),
                        in_=ot[:rows, :],
                    )

    nc.compile()
    return nc


def kernel(x: np.ndarray) -> np.ndarray:
    global _compiled
    from concourse.bass_utils import run_bass_kernel_spmd

    if _compiled is None:
        _compiled = _build()
    nc = _compiled

    x = np.ascontiguousarray(x, dtype=np.float32)
    in_maps = [{"x": x[c * BPC:(c + 1) * BPC]} for c in range(NCORES)]
    res = run_bass_kernel_spmd(nc, in_maps, list(range(NCORES)))
    out = np.concatenate([res.results[c]["out"] for c in range(NCORES)], axis=0)
    return out.astype(np.float32)
